# revision 6
# baseline (speedup 1.0000x reference)
import numpy as np
import ml_dtypes

import concourse.bacc as bacc
import concourse.mybir as mybir
import concourse.tile as tile
from concourse import bass_utils

N_NODES = 50000
N_EDGES = 800000
F = 64
NC = 8
RPC = N_NODES // NC          # rows per core
BLK = 128
NBLK = (RPC + BLK - 1) // BLK  # 49
NPAD = NBLK * BLK            # 6272 padded rows per core
CH = 512                     # edges per chunk
GPC = CH // BLK              # groups per chunk = 4

F32 = mybir.dt.float32
BF16 = mybir.dt.bfloat16
NP_BF16 = ml_dtypes.bfloat16
_last_res = None


def _build(G_b, mmdt):
    """Build the SPMD bass program. G_b: groups per block (len NBLK)."""
    TG = int(np.sum(G_b))
    TC = TG // GPC
    Ep = TG * BLK
    # static group -> block map + first/last flags
    blk_of, first_g, last_g = [], [], []
    for b in range(NBLK):
        for j in range(G_b[b]):
            blk_of.append(b)
            first_g.append(j == 0)
            last_g.append(j == G_b[b] - 1)

    nc = bacc.Bacc("TRN2", target_bir_lowering=False, debug=False,
                   num_devices=NC)

    x_d = nc.dram_tensor("x", [128, Ep], mmdt, kind="ExternalInput")
    cv_d = nc.dram_tensor("cv", [TC, 128, GPC, 16], F32, kind="ExternalInput")
    rl_d = nc.dram_tensor("rl", [TC, 128, GPC], F32, kind="ExternalInput")
    hown_d = nc.dram_tensor("hown", [F, NPAD], mmdt, kind="ExternalInput")
    w1h_d = nc.dram_tensor("w1h", [128, F], mmdt, kind="ExternalInput")
    w1r_d = nc.dram_tensor("w1r", [3, F], mmdt, kind="ExternalInput")
    w2_d = nc.dram_tensor("w2", [F, F], mmdt, kind="ExternalInput")
    cw1_d = nc.dram_tensor("cw1", [F, F], mmdt, kind="ExternalInput")
    cw_d = nc.dram_tensor("cw", [F, 1], mmdt, kind="ExternalInput")
    nw1a_d = nc.dram_tensor("nw1a", [F, F], mmdt, kind="ExternalInput")
    nw1b_d = nc.dram_tensor("nw1b", [F, F], mmdt, kind="ExternalInput")
    nw2_d = nc.dram_tensor("nw2", [F, F], mmdt, kind="ExternalInput")
    be1_d = nc.dram_tensor("be1", [F, 1], F32, kind="ExternalInput")
    be2_d = nc.dram_tensor("be2", [F, 1], F32, kind="ExternalInput")
    bc1_d = nc.dram_tensor("bc1", [F, 1], F32, kind="ExternalInput")
    bn1_d = nc.dram_tensor("bn1", [F, 1], F32, kind="ExternalInput")
    bn2_d = nc.dram_tensor("bn2", [F, 1], F32, kind="ExternalInput")
    iota_d = nc.dram_tensor("iota", [128, 128], F32, kind="ExternalInput")
    id128f_d = nc.dram_tensor("id128f", [128, 128], F32, kind="ExternalInput")
    id64m_d = nc.dram_tensor("id64m", [F, F], mmdt, kind="ExternalInput")
    id64f_d = nc.dram_tensor("id64f", [F, F], F32, kind="ExternalInput")

    hout_d = nc.dram_tensor("hout", [NPAD, F], F32, kind="ExternalOutput")
    cout_d = nc.dram_tensor("cout", [NPAD, 4], F32, kind="ExternalOutput")

    Relu = mybir.ActivationFunctionType.Relu
    Ident = mybir.ActivationFunctionType.Identity
    Sqrt = mybir.ActivationFunctionType.Sqrt
    mult = mybir.AluOpType.mult
    subtract = mybir.AluOpType.subtract
    is_equal = mybir.AluOpType.is_equal

    with tile.TileContext(nc) as tc:
        with tc.tile_pool(name="const", bufs=1) as cp, \
             tc.tile_pool(name="io", bufs=3) as iop, \
             tc.tile_pool(name="work", bufs=2) as wp, \
             tc.tile_pool(name="mlp", bufs=2) as mp, \
             tc.tile_pool(name="pmlp", bufs=2, space="PSUM") as pmlp, \
             tc.tile_pool(name="paux", bufs=3, space="PSUM") as paux, \
             tc.tile_pool(name="pagg", bufs=2, space="PSUM") as pagg, \
             tc.tile_pool(name="pnode", bufs=1, space="PSUM") as pnode:

            def cload(d, shape, dt, tag):
                t = cp.tile(shape, dt, tag=tag)
                nc.sync.dma_start(t[:], d.ap())
                return t

            w1h = cload(w1h_d, [128, F], mmdt, "w1h")
            w1r = cload(w1r_d, [3, F], mmdt, "w1r")
            w2 = cload(w2_d, [F, F], mmdt, "w2")
            cw1 = cload(cw1_d, [F, F], mmdt, "cw1")
            cw = cload(cw_d, [F, 1], mmdt, "cw")
            nw1a = cload(nw1a_d, [F, F], mmdt, "nw1a")
            nw1b = cload(nw1b_d, [F, F], mmdt, "nw1b")
            nw2 = cload(nw2_d, [F, F], mmdt, "nw2")
            be1 = cload(be1_d, [F, 1], F32, "be1")
            be2 = cload(be2_d, [F, 1], F32, "be2")
            bc1 = cload(bc1_d, [F, 1], F32, "bc1")
            bn1 = cload(bn1_d, [F, 1], F32, "bn1")
            bn2 = cload(bn2_d, [F, 1], F32, "bn2")
            iota = cload(iota_d, [128, 128], F32, "iota")
            id128f = cload(id128f_d, [128, 128], F32, "id128f")
            id64m = cload(id64m_d, [F, F], mmdt, "id64m")
            id64f = cload(id64f_d, [F, F], F32, "id64f")
            hown = cload(hown_d, [F, NPAD], mmdt, "hown")

            cur_agg = {}

            def node_post(b, agg_ps):
                agg_sb = wp.tile([128, 68], F32, tag="agg_sb")
                nc.vector.tensor_copy(agg_sb[:], agg_ps[:])
                # coord out: seg / max(cnt,1)
                cnt = wp.tile([128, 1], F32, tag="cnt")
                nc.vector.tensor_scalar_max(cnt[:], agg_sb[:, 67:68], 1.0)
                rcp = wp.tile([128, 1], F32, tag="rcp")
                nc.vector.reciprocal(rcp[:], cnt[:])
                csb = wp.tile([128, 4], F32, tag="csb")
                nc.vector.tensor_tensor(
                    out=csb[:, 0:3], in0=agg_sb[:, 64:67],
                    in1=rcp[:].to_broadcast([128, 3]), op=mult)
                nc.gpsimd.memset(csb[:, 3:4], 0.0)
                nc.sync.dma_start(cout_d.ap()[b * BLK:(b + 1) * BLK, :], csb[:])
                # node MLP
                aggT_ps = pnode.tile([128, 128], F32, tag="node")
                nc.tensor.transpose(aggT_ps[0:64, :], agg_sb[:, 0:64], id128f[:])
                aggT = wp.tile([F, 128], mmdt, tag="aggT_sb")
                nc.scalar.copy(aggT[:], aggT_ps[0:64, :])
                nps = pnode.tile([128, 128], F32, tag="node")
                nc.tensor.matmul(nps[0:64, :], nw1a[:],
                                 hown[:, b * BLK:(b + 1) * BLK],
                                 start=True, stop=False)
                nc.tensor.matmul(nps[0:64, :], nw1b[:], aggT[:],
                                 start=False, stop=True)
                nh = wp.tile([F, 128], mmdt, tag="nh")
                nc.scalar.activation(nh[:], nps[0:64, :], Relu, bias=be_ap(bn1))
                hps = pnode.tile([128, 128], F32, tag="node")
                nc.tensor.matmul(hps[0:64, :], nw2[:], nh[:], start=True, stop=True)
                hfm = wp.tile([F, 128], F32, tag="hfm")
                nc.scalar.activation(hfm[:], hps[0:64, :], Ident, bias=be_ap(bn2))
                hT_ps = pnode.tile([128, 128], F32, tag="node")
                nc.tensor.transpose(hT_ps[:, 0:64], hfm[:], id64f[:])
                hT = wp.tile([128, F], F32, tag="hT_sb")
                nc.vector.tensor_copy(hT[:], hT_ps[:, 0:64])
                nc.sync.dma_start(hout_d.ap()[b * BLK:(b + 1) * BLK, :], hT[:])

            def be_ap(t):
                return t[:, 0:1]

            for c in range(TC):
                x_t = iop.tile([128, CH], mmdt, tag="x")
                nc.sync.dma_start(x_t[:], x_d.ap()[:, c * CH:(c + 1) * CH])
                cv = iop.tile([128, GPC, 16], F32, tag="cv")
                nc.sync.dma_start(cv[:], cv_d.ap()[c, :, :, :])
                rl = iop.tile([128, GPC], F32, tag="rl")
                nc.sync.dma_start(rl[:], rl_d.ap()[c, :, :])

                # ---- radial (edge-major) ----
                d6 = wp.tile([128, GPC, 6], F32, tag="d6")
                nc.vector.tensor_tensor(out=d6[:], in0=cv[:, :, 0:6],
                                        in1=cv[:, :, 6:12], op=subtract)
                sq = wp.tile([128, GPC, 6], F32, tag="sq")
                nc.vector.tensor_tensor(out=sq[:], in0=d6[:], in1=d6[:], op=mult)
                dotp = wp.tile([128, GPC, 3], F32, tag="dotp")
                nc.vector.tensor_tensor(out=dotp[:], in0=d6[:, :, 0:3],
                                        in1=d6[:, :, 3:6], op=mult)
                rq = wp.tile([128, GPC, 4], F32, tag="rq")
                nc.vector.tensor_reduce(rq[:, :, 0:1], sq[:, :, 0:3],
                                        mybir.AxisListType.X, mybir.AluOpType.add)
                nc.vector.tensor_reduce(rq[:, :, 1:2], sq[:, :, 3:6],
                                        mybir.AxisListType.X, mybir.AluOpType.add)
                nc.vector.tensor_reduce(rq[:, :, 2:3], dotp[:],
                                        mybir.AxisListType.X, mybir.AluOpType.add)
                rad = wp.tile([128, GPC, 3], F32, tag="rad")
                nc.scalar.activation(rad[:, :, 0:2], rq[:, :, 0:2], Sqrt)
                dn2 = wp.tile([128, GPC, 1], F32, tag="dn2")
                nc.vector.tensor_tensor(out=dn2[:], in0=rq[:, :, 0:1],
                                        in1=rq[:, :, 1:2], op=mult)
                dn = wp.tile([128, GPC, 1], F32, tag="dn")
                nc.scalar.activation(dn[:], dn2[:], Sqrt)
                dni = wp.tile([128, GPC, 1], F32, tag="dni")
                nc.vector.reciprocal(dni[:], dn[:])
                nc.vector.tensor_tensor(out=rad[:, :, 2:3], in0=rq[:, :, 2:3],
                                        in1=dni[:], op=mult)

                # radial -> feat-major [3, CH] via PE transposes
                radT_ps = paux.tile([3, CH], F32, tag="aux")
                for g in range(GPC):
                    nc.tensor.transpose(radT_ps[:, g * BLK:(g + 1) * BLK],
                                        rad[:, g, :], id128f[:])
                radT = wp.tile([3, CH], mmdt, tag="radT")
                nc.scalar.copy(radT[:], radT_ps[:])

                # ---- edge MLP (feat-major) ----
                ps1 = pmlp.tile([F, CH], F32, tag="mm")
                nc.tensor.matmul(ps1[:], w1h[:], x_t[:], start=True, stop=False)
                nc.tensor.matmul(ps1[:], w1r[:], radT[:], start=False, stop=True)
                m1 = mp.tile([F, CH], mmdt, tag="m1")
                nc.scalar.activation(m1[:], ps1[:], Relu, bias=be_ap(be1))
                ps2 = pmlp.tile([F, CH], F32, tag="mm")
                nc.tensor.matmul(ps2[:], w2[:], m1[:], start=True, stop=True)
                m2 = mp.tile([F, CH], mmdt, tag="m2")
                nc.scalar.activation(m2[:], ps2[:], Relu, bias=be_ap(be2))
                psc = pmlp.tile([F, CH], F32, tag="mm")
                nc.tensor.matmul(psc[:], cw1[:], m2[:], start=True, stop=True)
                mc = mp.tile([F, CH], mmdt, tag="mc")
                nc.scalar.activation(mc[:], psc[:], Relu, bias=be_ap(bc1))

                # ---- per-edge scalar s (edge-major) ----
                s_ps = paux.tile([128, GPC, 1], F32, tag="aux")
                for g in range(GPC):
                    nc.tensor.matmul(s_ps[:, g, :],
                                     mc[:, g * BLK:(g + 1) * BLK], cw[:],
                                     start=True, stop=True)

                # ---- m -> edge-major + scatter rhs ----
                mT_ps = paux.tile([128, GPC, F], mmdt, tag="aux")
                for g in range(GPC):
                    nc.tensor.transpose(mT_ps[:, g, :],
                                        m2[:, g * BLK:(g + 1) * BLK], id64m[:])
                srhs = wp.tile([128, GPC, 68], mmdt, tag="srhs")
                for g in range(GPC):
                    nc.vector.tensor_copy(srhs[:, g, 0:64], mT_ps[:, g, :])
                nc.vector.tensor_tensor(
                    out=srhs[:, :, 64:67], in0=d6[:, :, 0:3],
                    in1=s_ps[:].to_broadcast([128, GPC, 3]), op=mult)
                nc.vector.tensor_scalar_min(srhs[:, :, 64:67],
                                            srhs[:, :, 64:67], 100.0)
                nc.vector.tensor_scalar_max(srhs[:, :, 64:67],
                                            srhs[:, :, 64:67], -100.0)
                nc.gpsimd.memset(srhs[:, :, 67:68], 1.0)

                # ---- selector + scatter matmuls ----
                sel = wp.tile([128, GPC, 128], mmdt, tag="sel")
                for g in range(GPC):
                    nc.vector.tensor_tensor(
                        out=sel[:, g, :],
                        in0=rl[:, g:g + 1].to_broadcast([128, 128]),
                        in1=iota[:], op=is_equal)
                for g in range(GPC):
                    gi = c * GPC + g
                    b = blk_of[gi]
                    if first_g[gi]:
                        cur_agg[b] = pagg.tile([128, 68], F32, tag="agg", name="agg")
                    nc.tensor.matmul(cur_agg[b][:], sel[:, g, :],
                                     srhs[:, g, :],
                                     start=first_g[gi], stop=last_g[gi],
                                     skip_group_check=True)
                    if last_g[gi]:
                        node_post(b, cur_agg.pop(b))

    nc.compile()
    return nc, TC, Ep


def _layout(row):
    """Shared static layout from edge destinations. Returns per-core edge
    order, slots, and shared G_b."""
    core = row // RPC
    orders, blks, cnts = [], [], []
    for c in range(NC):
        idx = np.nonzero(core == c)[0]
        rl = row[idx] - c * RPC
        o = np.argsort(rl, kind="stable")
        idx = idx[o]
        rl = rl[o]
        b = rl // BLK
        orders.append(idx)
        blks.append(b)
        cnts.append(np.bincount(b, minlength=NBLK))
    cnt_max = np.max(np.stack(cnts), axis=0)
    G_b = np.maximum((cnt_max + BLK - 1) // BLK, 1).astype(np.int64)
    TG = int(G_b.sum())
    G_b[-1] += (-TG) % GPC
    off = np.concatenate([[0], np.cumsum(G_b * BLK)])
    slots = []
    for c in range(NC):
        b = blks[c]
        n = len(b)
        starts = np.concatenate([[0], np.cumsum(cnts[c])])[:-1]
        within = np.arange(n) - np.repeat(starts, cnts[c])
        slots.append(off[b] + within)
    return orders, slots, G_b


def kernel(h, coord, vel, edge_index,
           e_w1, e_b1, e_w2, e_b2,
           n_w1, n_b1, n_w2, n_b2,
           c_w1, c_b1, c_w):
    h = np.asarray(h, np.float32)
    coord = np.asarray(coord, np.float32)
    vel = np.asarray(vel, np.float32)
    ei = np.asarray(edge_index)
    row, col = ei[0].astype(np.int64), ei[1].astype(np.int64)
    e_w1 = np.asarray(e_w1, np.float32); e_b1 = np.asarray(e_b1, np.float32)
    e_w2 = np.asarray(e_w2, np.float32); e_b2 = np.asarray(e_b2, np.float32)
    n_w1 = np.asarray(n_w1, np.float32); n_b1 = np.asarray(n_b1, np.float32)
    n_w2 = np.asarray(n_w2, np.float32); n_b2 = np.asarray(n_b2, np.float32)
    c_w1 = np.asarray(c_w1, np.float32); c_b1 = np.asarray(c_b1, np.float32)
    c_w = np.asarray(c_w, np.float32)

    mmdt, np_mm = BF16, NP_BF16

    orders, slots, G_b = _layout(row)
    nc, TC, Ep = _build(G_b, mmdt)

    wshared = {
        "w1h": e_w1[0:128].astype(np_mm),
        "w1r": e_w1[128:131].astype(np_mm),
        "w2": e_w2.astype(np_mm),
        "cw1": c_w1.astype(np_mm),
        "cw": c_w.astype(np_mm),
        "nw1a": n_w1[0:64].astype(np_mm),
        "nw1b": n_w1[64:128].astype(np_mm),
        "nw2": n_w2.astype(np_mm),
        "be1": e_b1.reshape(F, 1), "be2": e_b2.reshape(F, 1),
        "bc1": c_b1.reshape(F, 1), "bn1": n_b1.reshape(F, 1),
        "bn2": n_b2.reshape(F, 1),
        "iota": np.broadcast_to(np.arange(128, dtype=np.float32), (128, 128)).copy(),
        "id128f": np.eye(128, dtype=np.float32),
        "id64m": np.eye(F).astype(np_mm),
        "id64f": np.eye(F, dtype=np.float32),
    }

    in_maps = []
    for c in range(NC):
        e = orders[c]; s = slots[c]
        r, cl = row[e], col[e]
        X = np.zeros((128, Ep), dtype=np_mm)
        X[0:64, s] = h[r].T.astype(np_mm)
        X[64:128, s] = h[cl].T.astype(np_mm)
        CV = np.zeros((Ep, 16), dtype=np.float32)
        CV[:, 0] = 1.0; CV[:, 3] = 1.0
        CV[s, 0:3] = coord[r]; CV[s, 3:6] = vel[r]
        CV[s, 6:9] = coord[cl]; CV[s, 9:12] = vel[cl]
        CV = CV.reshape(TC, GPC, 128, 16).transpose(0, 2, 1, 3).copy()
        RL = np.full(Ep, 255.0, dtype=np.float32)
        RL[s] = ((row[e] - c * RPC) % BLK).astype(np.float32)
        RL = RL.reshape(TC, GPC, 128).transpose(0, 2, 1).copy()
        HO = np.zeros((F, NPAD), dtype=np_mm)
        HO[:, 0:RPC] = h[c * RPC:(c + 1) * RPC].T.astype(np_mm)
        im = {"x": X, "cv": CV, "rl": RL, "hown": HO}
        im.update(wshared)
        in_maps.append(im)

    res = bass_utils.run_bass_kernel_spmd(nc, in_maps,
                                          core_ids=list(range(NC)),
                                          trace=False)
    global _last_res
    _last_res = res
    h_out = np.empty((N_NODES, F), dtype=np.float32)
    c_out = np.empty((N_NODES, 3), dtype=np.float32)
    for c in range(NC):
        h_out[c * RPC:(c + 1) * RPC] = res.results[c]["hout"][0:RPC]
        c_out[c * RPC:(c + 1) * RPC] = res.results[c]["cout"][0:RPC, 0:3]
    return (h_out, c_out)
